# revision 26
# baseline (speedup 1.0000x reference)
"""MixGARCH Trainium2 kernel — unroll-by-4 linear-scan architecture.

Reference semantics: scan over t of
    v_t = relu(bias + Wx @ o_t^2 + Wh * v_{t-1}) + 1e-6,  hist[t] = v_t
with bias, Wx, Wh, o^2, v0 all >= 0, so relu is an identity and this is a
LINEAR first-order recurrence:
    v_t = Wh * v_{t-1} + c_t,   c_t = (bias + 1e-6) + Wx @ o_t^2

Unrolled by U=4, the recurrence at stride 4 is
    V_i = v_{t0+4i} = w^4 * V_{i-1} + d_i,
    d_i = sum_{m=0..3} w^m c_{t0+4i-m}
and the intermediate phases j=1..3 are
    v_{t0+4i+j} = sum_{m=0..j-1} w^m c_{t0+4i+j-m} + w^j V_i.

Mapping to engines (per core, halves stacked on partitions):
 - PE computes d (one matmul: 68 input rows = 2 halves x (8 ch x 4 lags +
   ones row for bias + init row for exact v0 injection)), and the phase
   reconstruction (partial-sum matmul over lagged inputs + diagonal w^j
   matmul against the scan output V, accumulated in one PSUM tile).
 - DVE runs tensor_tensor_scan (data1 read straight from PSUM) over TU
   columns only (T/4); phase tiles are copied PSUM->SBUF fp16 by ACT/DVE.
 - All DMA traffic is fp16.

Every half runs W=2048 warmup steps (w<0.9 ⇒ (w^4)^512 == 0.0f). Core 0
half 0's timeline starts at t0=-2048 with all-zero inputs, so its state is
exactly 0 at t=0 where the init row injects c_0 + w*vars0 for an exact
start. Phase outputs for the warmup window are never computed.
"""

import numpy as np
import ml_dtypes

BF16 = ml_dtypes.bfloat16

T = 524288
K = 64
NJ = 8
NCORES = 8
HALF = 32768
W = 2048              # warmup steps (real time) per half
U = 4                 # unroll factor
TU = (HALF + W) // U  # 8704 scan columns per half-timeline
XCOLS = TU + 1        # x2ph columns (partials read one column ahead)
F = 512               # window (PSUM tile) width
NW = TU // F          # 17 (window 0 = warmup only)
WU = W // U           # 512 = warmup columns
OC = TU - WU          # 8192 output columns per tensor
GRP = 4               # phase windows per output DMA group

_CACHE = {}


# ---------------------------------------------------------------------------
# Host-side packing
# ---------------------------------------------------------------------------

def _weights(bias, Wx, Wh, vars0):
    """Build the [128, 768] fp16 stationary-weight pack (shared by cores).

    Column blocks of 128: LTd | LTpA | LTpB | LTpC | LTq12 | LTq3.
    Row layout (contraction partitions): half h at base h*34:
      rows +m*8+n : o^2 channel n at lag m   (m=0..3)
      row  +32    : ones (bias)
      row  +33    : init (exact-v0 injection; used by core0 h0 only)
    """
    w = Wh.astype(np.float64)
    b = (bias.astype(np.float64) + 1e-6)
    Wxd = Wx.astype(np.float64)
    v0 = vars0.astype(np.float64)
    wp = [w**m for m in range(5)]  # wp[m] = w^m

    wts = np.zeros((128, 896), dtype=np.float64)
    for h in range(2):
        hb = h * 34
        oc = h * 64
        # --- LTd (cols 0:128): d_i = sum_m w^m c_{4i-m}
        for m in range(4):
            for n in range(NJ):
                wts[hb + m * 8 + n, 0 + oc:0 + oc + K] = wp[m] * Wxd[:, n]
        wts[hb + 32, 0 + oc:0 + oc + K] = b * (wp[0] + wp[1] + wp[2] + wp[3])
        wts[hb + 33, 0 + oc:0 + oc + K] = w * v0 - b * (wp[1] + wp[2] + wp[3])
        # --- Pj (cols 128j:128j+128), j=1..3: phase-j partial, block-diag
        # over halves (out col h*64+k), reading x2ph column i+1.
        jo = h * 64
        for j in (1, 2, 3):
            pc = 128 * j
            for mp in range(4 - j, 4):      # m' = 4-j .. 3
                coef = wp[mp - (4 - j)]     # w^(m'-(4-j))
                for n in range(NJ):
                    wts[hb + mp * 8 + n, pc + jo:pc + jo + K] = coef * Wxd[:, n]
            wts[hb + 32, pc + jo:pc + jo + K] = b * sum(wp[m] for m in range(j))
    # --- Qj (cols 512+128(j-1)): diagonal w^j for both halves, (h,k) rows.
    for j in (1, 2, 3):
        qc = 512 + 128 * (j - 1)
        for r0 in (0, 64):
            for k in range(K):
                wts[r0 + k, qc + r0 + k] = wp[j][k]
    return wts.astype(BF16)


def _host_prep(series, vars0, bias, Wx, Wh):
    series = np.asarray(series, dtype=np.float32)
    vars0 = np.asarray(vars0, dtype=np.float32)
    bias = np.asarray(bias, dtype=np.float32)
    Wx = np.asarray(Wx, dtype=np.float32)
    Wh = np.asarray(Wh, dtype=np.float32)

    PAD = W + 4  # zero region below t=0 (core0 h0 timeline) + lag margin
    sq = np.zeros((PAD + T + 8, NJ), dtype=np.float32)
    sq[PAD:PAD + T] = series * series
    sqh = sq.astype(BF16)

    wts = _weights(bias, Wx, Wh, vars0)
    wpow = Wh.astype(np.float64)
    ws = np.zeros((128, F + 4), dtype=np.float32)
    for r0 in (0, 64):
        ws[r0:r0 + 64, 0:F] = (wpow ** 4).astype(np.float32)[:, None]
        for j in (1, 2, 3):
            ws[r0:r0 + 64, F + j - 1] = (wpow ** j).astype(np.float32)

    in_maps = []
    for c in range(NCORES):
        x2 = np.zeros((68, XCOLS), dtype=BF16)
        for h in range(2):
            hb = h * 34
            t0 = c * 65536 + h * HALF - W
            for m in range(4):
                s = t0 - m + PAD
                for n in range(NJ):
                    x2[hb + m * 8 + n, :] = sqh[s:s + 4 * XCOLS:4, n]
            # ones row: active only where the timeline is in real time
            # (t0 + 4i >= 0); for core0 h0 that's i >= WU.
            io = 0 if t0 >= 0 else WU
            x2[hb + 32, io:] = 1.0
            if c == 0 and h == 0:
                x2[hb + 33, WU] = 1.0
        in_maps.append({"x2ph": x2, "wts": wts, "wscan": ws})
    return in_maps


def _assemble(results):
    hist = np.empty((T, K), dtype=np.float32)
    for c in range(NCORES):
        vv = results[c]["vout_v"].astype(np.float32)
        vph = results[c]["vout_ph"].astype(np.float32)
        va, vb, vc = vph[:, 0], vph[:, 1], vph[:, 2]
        for h in range(2):
            hs = c * 65536 + h * HALF
            r0 = h * 64
            hist[hs + 0:hs + HALF:4, :] = vv[r0:r0 + 64, :].T
            hist[hs + 1:hs + HALF:4, :] = va[r0:r0 + 64, :].T
            hist[hs + 2:hs + HALF:4, :] = vb[r0:r0 + 64, :].T
            hist[hs + 3:hs + HALF:4, :] = vc[r0:r0 + 64, :].T
    return hist


# ---------------------------------------------------------------------------
# Numpy emulation of the device dataflow (validation aid)
# ---------------------------------------------------------------------------

def _emulate(inputs):
    """Emulate the device kernel in numpy (fp32 accumulation, fp16
    storage) using the exact packed tensors; returns assembled hist."""
    in_maps = _host_prep(
        inputs["series"], inputs["vars0"], inputs["bias"],
        inputs["Wx"], inputs["Wh"],
    )
    results = []
    for c in range(NCORES):
        x2 = in_maps[c]["x2ph"].astype(np.float32)
        wts = in_maps[c]["wts"].astype(np.float32)
        ws = in_maps[c]["wscan"].astype(np.float32)
        d = wts[0:68, 0:128].T @ x2  # stays fp32 (PSUM)
        # Scan state stays fp32 across steps; only out is fp16.
        state = np.zeros(128, dtype=np.float32)
        svf = np.empty((128, TU), dtype=np.float32)
        for i in range(TU):
            state = ws[:, 0] * state + d[:, i]
            svf[:, i] = state
        sv = svf.astype(BF16)
        svf32 = sv.astype(np.float32)
        va = wts[0:68, 128:256].T @ x2[:, 1:] + ws[:, F:F + 1] * svf32
        vb = wts[0:68, 256:384].T @ x2[:, 1:] + ws[:, F + 1:F + 2] * svf32
        vc = wts[0:68, 384:512].T @ x2[:, 1:] + ws[:, F + 2:F + 3] * svf32
        # va[:, i] corresponds to timeline column i (partial reads x2 col
        # i+1, diag reads sv col i); output columns are [WU, TU).
        results.append({
            "vout_v": sv[:, WU:],
            "vout_ph": np.stack(
                [va[:, WU:], vb[:, WU:], vc[:, WU:]], axis=1).astype(BF16),
        })
    return _assemble(results)


# ---------------------------------------------------------------------------
# Bass kernel
# ---------------------------------------------------------------------------

def _build_nc():
    import concourse.bacc as bacc
    import concourse.mybir as mybir
    import concourse.tile as tile

    f32 = mybir.dt.float32
    f16 = mybir.dt.bfloat16

    nc = bacc.Bacc(None, target_bir_lowering=False)
    x2d = nc.dram_tensor("x2ph", [68, XCOLS], f16, kind="ExternalInput")
    wtd = nc.dram_tensor("wts", [128, 896], f16, kind="ExternalInput")
    wsd = nc.dram_tensor("wscan", [128, F + 4], f32, kind="ExternalInput")
    vv = nc.dram_tensor("vout_v", [128, OC], f16, kind="ExternalOutput")
    vp = nc.dram_tensor("vout_ph", [128, 3, OC], f16, kind="ExternalOutput")

    AF = mybir.ActivationFunctionType
    ALU = mybir.AluOpType

    with tile.TileContext(nc) as tc:
        with (
            tc.tile_pool(name="const", bufs=1) as cpool,
            tc.tile_pool(name="big", bufs=1) as bpool,
            tc.tile_pool(name="stage", bufs=3) as stpool,
            tc.tile_pool(name="psum", bufs=1, space="PSUM") as ps,
        ):
            wt_sb = cpool.tile([128, 896], f16)
            nc.scalar.dma_start(wt_sb[:, 0:128], wtd[:, 0:128])
            nc.scalar.dma_start(wt_sb[:, 128:896], wtd[:, 128:896])
            ws_sb = cpool.tile([128, F + 4], f32)
            nc.scalar.dma_start(ws_sb[:], wsd[:])

            x2_sb = bpool.tile([68, XCOLS], f16)
            XCH = [(0, 1153), (1153, 1536), (2689, 2048),
                   (4737, 2048), (6785, 1920)]
            for s, n in XCH:
                nc.sync.dma_start(x2_sb[:, s:s + n], x2d[:, s:s + n])

            sv_sb = bpool.tile([128, TU], f16)

            def emit_d_scan(w):
                # Window 0 is pure warmup: 128 columns wash in the state
                # ((w^4)^128 == 0f), so skip its first 384 columns.
                win = slice(w * F + (384 if w == 0 else 0), (w + 1) * F)
                fd = win.stop - win.start
                ps_d = ps.tile([128, F], f32, tag=f"d{w % 2}")
                nc.tensor.matmul(
                    ps_d[:, 0:fd], wt_sb[0:68, 0:128], x2_sb[0:68, win],
                    start=True, stop=True,
                )
                initial = 0.0 if w == 0 else sv_sb[:, w * F - 1:w * F]
                nc.vector.tensor_tensor_scan(
                    sv_sb[:, win], ws_sb[:, 0:fd], ps_d[:, 0:fd], initial,
                    ALU.mult, ALU.add,
                )

            pps = {}

            def emit_partials(w):
                win1 = slice(w * F + 1, (w + 1) * F + 1)
                tiles = []
                for j in (1, 2, 3):
                    ps_j = ps.tile([128, F], f32, tag=f"p{j}{w % 2}")
                    stop = FUSED[j](w)  # no diag accumulation when fused
                    nc.tensor.matmul(ps_j[:], wt_sb[0:68, 128 * j:128 * j + 128],
                                     x2_sb[0:68, win1], start=True, stop=stop)
                    tiles.append(ps_j)
                pps[w] = tiles

            # fused-on-DVE schedule per phase: j=3 always, j=2 three of four
            # windows, j=1 never (diag matmul + ACT copy instead).
            FUSED = {1: lambda w: False,
                     2: lambda w: w % 4 == 1,
                     3: lambda w: True}

            stg = []
            emit_d_scan(0)
            emit_d_scan(1)
            emit_partials(1)
            emit_partials(2)
            for w in range(1, NW):
                if w + 1 < NW:
                    emit_d_scan(w + 1)
                win = slice(w * F, (w + 1) * F)
                g, gi = (w - 1) // GRP, (w - 1) % GRP
                GF = GRP * F
                if gi == 0:
                    stg = []
                    for j in (1, 2, 3):
                        stg_t = stpool.tile([128, GF], f16, tag=f"g{j}")
                        stg.append(stg_t)
                ssl = slice(gi * F, (gi + 1) * F)

                tiles = pps.pop(w)
                for j in (1, 2, 3):
                    ps_j = tiles[j - 1]
                    if FUSED[j](w):
                        nc.vector.scalar_tensor_tensor(
                            stg[j - 1][:, ssl], sv_sb[:, win],
                            ws_sb[:, F + j - 1:F + j], ps_j[:],
                            ALU.mult, ALU.add,
                        )
                    else:
                        qc = 512 + 128 * (j - 1)
                        nc.tensor.matmul(ps_j[:], wt_sb[0:128, qc:qc + 128],
                                         sv_sb[:, win], start=False, stop=True)
                        nc.scalar.activation(stg[j - 1][:, ssl], ps_j[:],
                                             AF.Identity)

                if w + 2 < NW:
                    emit_partials(w + 2)

                if g < 3:
                    if gi == GRP - 1:
                        for j in (1, 2, 3):
                            nc.sync.dma_start(
                                vp[:, j - 1, g * GF:(g + 1) * GF],
                                stg[j - 1][:]
                            )
                else:
                    # last group: drain in 2+1+1 window chunks, spreading
                    # the final writes across all three DMA issue queues
                    if gi >= 1:
                        lo = 0 if gi == 1 else gi * F
                        qs = {1: (nc.sync, nc.sync, nc.sync),
                              2: (nc.sync, nc.scalar, nc.sync),
                              3: (nc.sync, nc.scalar, nc.gpsimd)}[gi]
                        for j in (1, 2, 3):
                            qs[j - 1].dma_start(
                                vp[:, j - 1, 6144 + lo:6144 + (gi + 1) * F],
                                stg[j - 1][:, lo:(gi + 1) * F]
                            )

                # V output DMA as the scan completes slices (last part split).
                if w in (4, 8, 12):
                    gq = w // GRP - 1
                    s = WU + gq * 2048
                    nc.sync.dma_start(vv[:, gq * 2048:(gq + 1) * 2048],
                                      sv_sb[:, s:s + 2048])
                elif w == 15:
                    nc.sync.dma_start(vv[:, 6144:7680], sv_sb[:, 6656:8192])
                elif w == 16:
                    nc.scalar.dma_start(vv[:, 7680:8192], sv_sb[:, 8192:8704])

    nc.compile()
    return nc


def run(inputs, trace=False, **kw):
    from concourse.bass_utils import run_bass_kernel_spmd

    if "nc" not in _CACHE:
        _CACHE["nc"] = _build_nc()
    nc = _CACHE["nc"]
    in_maps = _host_prep(
        inputs["series"], inputs["vars0"], inputs["bias"],
        inputs["Wx"], inputs["Wh"],
    )
    res = run_bass_kernel_spmd(
        nc, in_maps, core_ids=list(range(NCORES)), trace=trace, **kw
    )
    return _assemble(res.results), res


def kernel(series, vars0, bias, Wx, Wh):
    out, _ = run(
        {"series": series, "vars0": vars0, "bias": bias, "Wx": Wx, "Wh": Wh}
    )
    return out


# revision 27
# speedup vs baseline: 1.0633x; 1.0633x over previous
"""MixGARCH Trainium2 kernel — unroll-by-4 linear-scan architecture.

Reference semantics: scan over t of
    v_t = relu(bias + Wx @ o_t^2 + Wh * v_{t-1}) + 1e-6,  hist[t] = v_t
with bias, Wx, Wh, o^2, v0 all >= 0, so relu is an identity and this is a
LINEAR first-order recurrence:
    v_t = Wh * v_{t-1} + c_t,   c_t = (bias + 1e-6) + Wx @ o_t^2

Unrolled by U=4, the recurrence at stride 4 is
    V_i = v_{t0+4i} = w^4 * V_{i-1} + d_i,
    d_i = sum_{m=0..3} w^m c_{t0+4i-m}
and the intermediate phases j=1..3 are
    v_{t0+4i+j} = sum_{m=0..j-1} w^m c_{t0+4i+j-m} + w^j V_i.

Mapping to engines (per core, halves stacked on partitions):
 - PE computes d (one matmul: 68 input rows = 2 halves x (8 ch x 4 lags +
   ones row for bias + init row for exact v0 injection)), and the phase
   reconstruction (partial-sum matmul over lagged inputs + diagonal w^j
   matmul against the scan output V, accumulated in one PSUM tile).
 - DVE runs tensor_tensor_scan (data1 read straight from PSUM) over TU
   columns only (T/4); phase tiles are copied PSUM->SBUF fp16 by ACT/DVE.
 - All DMA traffic is fp16.

Every half runs W=2048 warmup steps (w<0.9 ⇒ (w^4)^512 == 0.0f). Core 0
half 0's timeline starts at t0=-2048 with all-zero inputs, so its state is
exactly 0 at t=0 where the init row injects c_0 + w*vars0 for an exact
start. Phase outputs for the warmup window are never computed.
"""

import numpy as np
import ml_dtypes

BF16 = ml_dtypes.bfloat16

T = 524288
K = 64
NJ = 8
NCORES = 8
HALF = 32768
W = 2048              # warmup steps (real time) per half
U = 4                 # unroll factor
TU = (HALF + W) // U  # 8704 scan columns per half-timeline
XCOLS = TU + 1        # x2ph columns (partials read one column ahead)
F = 512               # window (PSUM tile) width
NW = TU // F          # 17 (window 0 = warmup only)
WU = W // U           # 512 = warmup columns
OC = TU - WU          # 8192 output columns per tensor
GRP = 4               # phase windows per output DMA group

_CACHE = {}


# ---------------------------------------------------------------------------
# Host-side packing
# ---------------------------------------------------------------------------

def _weights(bias, Wx, Wh, vars0):
    """Build the [128, 768] fp16 stationary-weight pack (shared by cores).

    Column blocks of 128: LTd | LTpA | LTpB | LTpC | LTq12 | LTq3.
    Row layout (contraction partitions): half h at base h*34:
      rows +m*8+n : o^2 channel n at lag m   (m=0..3)
      row  +32    : ones (bias)
      row  +33    : init (exact-v0 injection; used by core0 h0 only)
    """
    w = Wh.astype(np.float64)
    b = (bias.astype(np.float64) + 1e-6)
    Wxd = Wx.astype(np.float64)
    v0 = vars0.astype(np.float64)
    wp = [w**m for m in range(5)]  # wp[m] = w^m

    wts = np.zeros((128, 896), dtype=np.float64)
    for h in range(2):
        hb = h * 34
        oc = h * 64
        # --- LTd (cols 0:128): d_i = sum_m w^m c_{4i-m}
        for m in range(4):
            for n in range(NJ):
                wts[hb + m * 8 + n, 0 + oc:0 + oc + K] = wp[m] * Wxd[:, n]
        wts[hb + 32, 0 + oc:0 + oc + K] = b * (wp[0] + wp[1] + wp[2] + wp[3])
        wts[hb + 33, 0 + oc:0 + oc + K] = w * v0 - b * (wp[1] + wp[2] + wp[3])
        # --- Pj (cols 128j:128j+128), j=1..3: phase-j partial, block-diag
        # over halves (out col h*64+k), reading x2ph column i+1.
        jo = h * 64
        for j in (1, 2, 3):
            pc = 128 * j
            for mp in range(4 - j, 4):      # m' = 4-j .. 3
                coef = wp[mp - (4 - j)]     # w^(m'-(4-j))
                for n in range(NJ):
                    wts[hb + mp * 8 + n, pc + jo:pc + jo + K] = coef * Wxd[:, n]
            wts[hb + 32, pc + jo:pc + jo + K] = b * sum(wp[m] for m in range(j))
    # --- Qj (cols 512+128(j-1)): diagonal w^j for both halves, (h,k) rows.
    for j in (1, 2, 3):
        qc = 512 + 128 * (j - 1)
        for r0 in (0, 64):
            for k in range(K):
                wts[r0 + k, qc + r0 + k] = wp[j][k]
    return wts.astype(BF16)


def _host_prep(series, vars0, bias, Wx, Wh):
    series = np.asarray(series, dtype=np.float32)
    vars0 = np.asarray(vars0, dtype=np.float32)
    bias = np.asarray(bias, dtype=np.float32)
    Wx = np.asarray(Wx, dtype=np.float32)
    Wh = np.asarray(Wh, dtype=np.float32)

    PAD = W + 4  # zero region below t=0 (core0 h0 timeline) + lag margin
    sq = np.zeros((PAD + T + 8, NJ), dtype=np.float32)
    sq[PAD:PAD + T] = series * series
    sqh = sq.astype(BF16)

    wts = _weights(bias, Wx, Wh, vars0)
    wpow = Wh.astype(np.float64)
    ws = np.zeros((128, F + 4), dtype=np.float32)
    for r0 in (0, 64):
        ws[r0:r0 + 64, 0:F] = (wpow ** 4).astype(np.float32)[:, None]
        for j in (1, 2, 3):
            ws[r0:r0 + 64, F + j - 1] = (wpow ** j).astype(np.float32)

    in_maps = []
    for c in range(NCORES):
        x2 = np.zeros((68, XCOLS), dtype=BF16)
        for h in range(2):
            hb = h * 34
            t0 = c * 65536 + h * HALF - W
            for m in range(4):
                s = t0 - m + PAD
                for n in range(NJ):
                    x2[hb + m * 8 + n, :] = sqh[s:s + 4 * XCOLS:4, n]
            # ones row: active only where the timeline is in real time
            # (t0 + 4i >= 0); for core0 h0 that's i >= WU.
            io = 0 if t0 >= 0 else WU
            x2[hb + 32, io:] = 1.0
            if c == 0 and h == 0:
                x2[hb + 33, WU] = 1.0
        in_maps.append({"x2ph": x2, "wts": wts, "wscan": ws})
    return in_maps


def _assemble(results):
    hist = np.empty((T, K), dtype=np.float32)
    for c in range(NCORES):
        vv = results[c]["vout_v"].astype(np.float32)
        vph = results[c]["vout_ph"].astype(np.float32)
        va, vb, vc = vph[:, 0], vph[:, 1], vph[:, 2]
        for h in range(2):
            hs = c * 65536 + h * HALF
            r0 = h * 64
            hist[hs + 0:hs + HALF:4, :] = vv[r0:r0 + 64, :].T
            hist[hs + 1:hs + HALF:4, :] = va[r0:r0 + 64, :].T
            hist[hs + 2:hs + HALF:4, :] = vb[r0:r0 + 64, :].T
            hist[hs + 3:hs + HALF:4, :] = vc[r0:r0 + 64, :].T
    return hist


# ---------------------------------------------------------------------------
# Numpy emulation of the device dataflow (validation aid)
# ---------------------------------------------------------------------------

def _emulate(inputs):
    """Emulate the device kernel in numpy (fp32 accumulation, fp16
    storage) using the exact packed tensors; returns assembled hist."""
    in_maps = _host_prep(
        inputs["series"], inputs["vars0"], inputs["bias"],
        inputs["Wx"], inputs["Wh"],
    )
    results = []
    for c in range(NCORES):
        x2 = in_maps[c]["x2ph"].astype(np.float32)
        wts = in_maps[c]["wts"].astype(np.float32)
        ws = in_maps[c]["wscan"].astype(np.float32)
        d = wts[0:68, 0:128].T @ x2  # stays fp32 (PSUM)
        # Scan state stays fp32 across steps; only out is fp16.
        state = np.zeros(128, dtype=np.float32)
        svf = np.empty((128, TU), dtype=np.float32)
        for i in range(TU):
            state = ws[:, 0] * state + d[:, i]
            svf[:, i] = state
        sv = svf.astype(BF16)
        svf32 = sv.astype(np.float32)
        va = wts[0:68, 128:256].T @ x2[:, 1:] + ws[:, F:F + 1] * svf32
        vb = wts[0:68, 256:384].T @ x2[:, 1:] + ws[:, F + 1:F + 2] * svf32
        vc = wts[0:68, 384:512].T @ x2[:, 1:] + ws[:, F + 2:F + 3] * svf32
        # va[:, i] corresponds to timeline column i (partial reads x2 col
        # i+1, diag reads sv col i); output columns are [WU, TU).
        results.append({
            "vout_v": sv[:, WU:],
            "vout_ph": np.stack(
                [va[:, WU:], vb[:, WU:], vc[:, WU:]], axis=1).astype(BF16),
        })
    return _assemble(results)


# ---------------------------------------------------------------------------
# Bass kernel
# ---------------------------------------------------------------------------

def _build_nc():
    import concourse.bacc as bacc
    import concourse.mybir as mybir
    import concourse.tile as tile

    f32 = mybir.dt.float32
    f16 = mybir.dt.bfloat16

    nc = bacc.Bacc(None, target_bir_lowering=False)
    x2d = nc.dram_tensor("x2ph", [68, XCOLS], f16, kind="ExternalInput")
    wtd = nc.dram_tensor("wts", [128, 896], f16, kind="ExternalInput")
    wsd = nc.dram_tensor("wscan", [128, F + 4], f32, kind="ExternalInput")
    vv = nc.dram_tensor("vout_v", [128, OC], f16, kind="ExternalOutput")
    vp = nc.dram_tensor("vout_ph", [128, 3, OC], f16, kind="ExternalOutput")

    AF = mybir.ActivationFunctionType
    ALU = mybir.AluOpType

    with tile.TileContext(nc) as tc:
        with (
            tc.tile_pool(name="const", bufs=1) as cpool,
            tc.tile_pool(name="big", bufs=1) as bpool,
            tc.tile_pool(name="stage", bufs=2) as stpool,
            tc.tile_pool(name="psum", bufs=1, space="PSUM") as ps,
        ):
            wt_sb = cpool.tile([128, 896], f16)
            nc.scalar.dma_start(wt_sb[:, 0:128], wtd[:, 0:128])
            nc.scalar.dma_start(wt_sb[:, 128:896], wtd[:, 128:896])
            ws_sb = cpool.tile([128, F + 4], f32)
            nc.scalar.dma_start(ws_sb[:], wsd[:])

            x2_sb = bpool.tile([68, XCOLS], f16)
            XCH = [(0, 1153), (1153, 1536), (2689, 2048),
                   (4737, 2048), (6785, 1920)]
            for s, n in XCH:
                nc.sync.dma_start(x2_sb[:, s:s + n], x2d[:, s:s + n])

            sv_sb = bpool.tile([128, TU], f16)

            def emit_d_scan(w):
                # Window 0 is pure warmup: 128 columns wash in the state
                # ((w^4)^128 == 0f), so skip its first 384 columns.
                win = slice(w * F + (384 if w == 0 else 0), (w + 1) * F)
                fd = win.stop - win.start
                ps_d = ps.tile([128, F], f32, tag=f"d{w % 2}")
                nc.tensor.matmul(
                    ps_d[:, 0:fd], wt_sb[0:68, 0:128], x2_sb[0:68, win],
                    start=True, stop=True,
                )
                initial = 0.0 if w == 0 else sv_sb[:, w * F - 1:w * F]
                nc.vector.tensor_tensor_scan(
                    sv_sb[:, win], ws_sb[:, 0:fd], ps_d[:, 0:fd], initial,
                    ALU.mult, ALU.add,
                )

            pps = {}

            def emit_partials(w):
                win1 = slice(w * F + 1, (w + 1) * F + 1)
                tiles = []
                for j in (1, 2, 3):
                    ps_j = ps.tile([128, F], f32, tag=f"p{j}{w % 2}")
                    stop = FUSED[j](w)  # no diag accumulation when fused
                    nc.tensor.matmul(ps_j[:], wt_sb[0:68, 128 * j:128 * j + 128],
                                     x2_sb[0:68, win1], start=True, stop=stop)
                    tiles.append(ps_j)
                pps[w] = tiles

            # fused-on-DVE schedule per phase: j=3 always, j=2 three of four
            # windows, j=1 never (diag matmul + ACT copy instead).
            FUSED = {1: lambda w: False,
                     2: lambda w: w % 4 == 1,
                     3: lambda w: True}

            stg = []
            emit_d_scan(0)
            emit_d_scan(1)
            emit_partials(1)
            emit_partials(2)
            for w in range(1, NW):
                if w + 1 < NW:
                    emit_d_scan(w + 1)
                win = slice(w * F, (w + 1) * F)
                g, gi = (w - 1) // GRP, (w - 1) % GRP
                GF = GRP * F
                if gi == 0:
                    stg = []
                    for j in (1, 2, 3):
                        stg_t = stpool.tile([128, GF], f16, tag=f"g{j}")
                        stg.append(stg_t)
                ssl = slice(gi * F, (gi + 1) * F)

                tiles = pps.pop(w)
                for j in (1, 2, 3):
                    ps_j = tiles[j - 1]
                    if FUSED[j](w):
                        nc.vector.scalar_tensor_tensor(
                            stg[j - 1][:, ssl], sv_sb[:, win],
                            ws_sb[:, F + j - 1:F + j], ps_j[:],
                            ALU.mult, ALU.add,
                        )
                    else:
                        qc = 512 + 128 * (j - 1)
                        nc.tensor.matmul(ps_j[:], wt_sb[0:128, qc:qc + 128],
                                         sv_sb[:, win], start=False, stop=True)
                        nc.scalar.activation(stg[j - 1][:, ssl], ps_j[:],
                                             AF.Identity)

                if w + 2 < NW:
                    emit_partials(w + 2)

                if g < 3:
                    if gi == GRP - 1:
                        for j in (1, 2, 3):
                            nc.sync.dma_start(
                                vp[:, j - 1, g * GF:(g + 1) * GF],
                                stg[j - 1][:]
                            )
                else:
                    # last group: drain in 2+1+1 window chunks, spreading
                    # the final writes across all three DMA issue queues
                    if gi >= 1:
                        lo = 0 if gi == 1 else gi * F
                        qs = {1: (nc.sync, nc.sync, nc.sync),
                              2: (nc.sync, nc.scalar, nc.sync),
                              3: (nc.sync, nc.scalar, nc.gpsimd)}[gi]
                        for j in (1, 2, 3):
                            qs[j - 1].dma_start(
                                vp[:, j - 1, 6144 + lo:6144 + (gi + 1) * F],
                                stg[j - 1][:, lo:(gi + 1) * F]
                            )

                # V output DMA as the scan completes slices (last part split).
                if w in (4, 8, 12):
                    gq = w // GRP - 1
                    s = WU + gq * 2048
                    nc.sync.dma_start(vv[:, gq * 2048:(gq + 1) * 2048],
                                      sv_sb[:, s:s + 2048])
                elif w == 15:
                    nc.sync.dma_start(vv[:, 6144:7680], sv_sb[:, 6656:8192])
                elif w == 16:
                    nc.scalar.dma_start(vv[:, 7680:8192], sv_sb[:, 8192:8704])

    nc.compile()
    return nc


def run(inputs, trace=False, **kw):
    from concourse.bass_utils import run_bass_kernel_spmd

    if "nc" not in _CACHE:
        _CACHE["nc"] = _build_nc()
    nc = _CACHE["nc"]
    in_maps = _host_prep(
        inputs["series"], inputs["vars0"], inputs["bias"],
        inputs["Wx"], inputs["Wh"],
    )
    res = run_bass_kernel_spmd(
        nc, in_maps, core_ids=list(range(NCORES)), trace=trace, **kw
    )
    return _assemble(res.results), res


def kernel(series, vars0, bias, Wx, Wh):
    out, _ = run(
        {"series": series, "vars0": vars0, "bias": bias, "Wx": Wx, "Wh": Wh}
    )
    return out
